# revision 15
# baseline (speedup 1.0000x reference)
"""BiLinearInteractionLayer (bilinear_type='all') Trainium2 Bass kernel.

Contract: kernel(inputs=[2048,40,64] f32, w=[64,64] f32) -> [2048, 49920] f32,
matching

    xw  = einsum('bfd,de->bfe', inputs, w)
    p   = xw[:, I, :] * inputs[:, J, :]   # (I, J) = triu_indices(40, k=1)
    out = p.reshape(B, -1)

Data-parallel over 8 NeuronCores: batch 2048 -> 8 x 256, W replicated.

v2 pipeline (per core, 2 x 128-row tiles):
  - x loads f32; ACT converts to bf16 for the PE path only
  - PE: bf16 transpose of each 2-field chunk (1-pass vs fp32's 2), then ONE
    bf16 matmul per chunk against a block-diag [[W,0],[0,W]] (f32 PSUM
    accumulate) -> xw chunk [128, 2*64] in one shot.  ~6x less PE time than
    the fp32 per-field scheme, so tile 1's xw is ready long before its muls.
  - pair muls (xw_i (x) v_j, f32, exact) split ~66/34 between DVE and Pool
    (gpsimd) so aggregate production rate stays above the ~410 GB/s DMA
    drain rate at all times.
  - ONE output DMA per 2-field chunk (40/core instead of 78): fewer
    semaphore updates (they ride DMA engine 79 and were the end-of-kernel
    straggler).
Only x and W are bf16-rounded inside the GEMM; the final elementwise product
is exact f32, so rel err ~1e-3 against the 2e-2 gate.
"""

import numpy as np
from contextlib import ExitStack

import concourse.bass as bass  # noqa: F401  (registers engines)
import concourse.bacc as bacc
import concourse.tile as tile
import concourse.mybir as mybir
from concourse.bass_utils import run_bass_kernel_spmd

B = 2048
F = 40
D = 64
NCORES = 8
BS = B // NCORES                   # 256 rows per core
PAIRS = F * (F - 1) // 2           # 780
OUT_W = PAIRS * D                  # 49920
FD = F * D                         # 2560
DT = mybir.dt.float32
BF = mybir.dt.bfloat16

BLOCK_LEN = [F - 1 - i for i in range(F - 1)]
BLOCK_OFF = np.concatenate([[0], np.cumsum(BLOCK_LEN)[:-1]]).tolist()

# chunk fp covers fields (2fp, 2fp+1); tail chunks (fields >= 30) first for
# tile 0 so the output stream starts as soon as the tail x DMA lands
SPLIT_F = 30
C0 = SPLIT_F * D                    # tail split column (f32 x)
TAIL_FPS = list(range(SPLIT_F // 2, F // 2))   # 15..19
HEAD_FPS = list(range(SPLIT_F // 2))           # 0..14

# phase-A (PE/ACT) chunk order, tile-sequential (interleaving the tiles
# measurably slowed both DVE and the HBM drain): t0 tail chunks first (ready
# as soon as the tail x DMA lands), then t0 heads, t1 heads, t1 tails
CHUNK_ORDER = (
    [(0, fp) for fp in TAIL_FPS]
    + [(0, fp) for fp in HEAD_FPS]
    + [(1, fp) for fp in HEAD_FPS]
    + [(1, fp) for fp in TAIL_FPS]
)

# phase-B (DVE mul + per-block DMA) production order: a small warmup burst
# from t0's tail fields (they only need the tail x chunk), then big blocks
# descending (production ~490 GB/s beats the ~418 GB/s drain, banking
# backlog in the stage ring), then t1's small tail blocks which drain out
# of the banked backlog.
BLOCK_ORDER = (
    [(0, i) for i in range(SPLIT_F, F - 1)]
    + [(0, i) for i in range(SPLIT_F)]
    + [(1, i) for i in range(SPLIT_F)]
    + [(1, i) for i in range(SPLIT_F, F - 1)]
)

_CACHE = {}


def _build(bs: int):
    assert bs % 128 == 0
    ntiles = bs // 128
    nc = bacc.Bacc("TRN2", target_bir_lowering=False, debug=False)

    x_dram = nc.dram_tensor("x", [bs, F, D], DT, kind="ExternalInput").ap()
    w_dram = nc.dram_tensor("w", [D, D], DT, kind="ExternalInput").ap()
    id_dram = nc.dram_tensor("ident", [128, 128], DT, kind="ExternalInput").ap()
    out_dram = nc.dram_tensor("out", [bs, OUT_W], DT, kind="ExternalOutput").ap()

    x_flat = x_dram.rearrange("b f d -> b (f d)")

    with tile.TileContext(nc) as tc, ExitStack() as ctx:
        const_pool = ctx.enter_context(tc.tile_pool(name="const", bufs=1))
        x_pool = ctx.enter_context(tc.tile_pool(name="x", bufs=2))
        xb_pool = ctx.enter_context(tc.tile_pool(name="xb", bufs=2))
        xw_pool = ctx.enter_context(tc.tile_pool(name="xw", bufs=2))
        tr_pool = ctx.enter_context(tc.tile_pool(name="tr", bufs=3))
        # one stage ring per output DMA queue; alternating blocks between two
        # queues hides each queue's slot-free -> mul -> issue chain latency
        # behind the other queue's drains.  Small blocks get their own deeper
        # rings (slots are max-size, so mixing them with jn=39 blocks wastes
        # SBUF and leaves the tail with too little elasticity).
        stage_a = ctx.enter_context(tc.tile_pool(name="stage_a", bufs=4))
        stage_b = ctx.enter_context(tc.tile_pool(name="stage_b", bufs=4))
        stage_as = ctx.enter_context(tc.tile_pool(name="stage_as", bufs=8))
        stage_bs = ctx.enter_context(tc.tile_pool(name="stage_bs", bufs=8))
        SMALL_JN = 15
        psum_tr = ctx.enter_context(tc.tile_pool(name="psum_tr", bufs=3, space="PSUM"))
        psum_mm = ctx.enter_context(tc.tile_pool(name="psum_mm", bufs=4, space="PSUM"))

        # ---- constants (sync queue: starts clean, lands earliest) ----
        ident = const_pool.tile([128, 128], DT)
        nc.sync.dma_start(ident[:], id_dram)
        ident_bf = const_pool.tile([128, 128], BF)
        nc.scalar.copy(ident_bf[:], ident[:])

        # f32 block-diag [[W,0],[0,W]] assembled by DMA into a zeroed tile,
        # then one ACT convert to bf16
        w_bdf = const_pool.tile([128, 128], DT)
        nc.gpsimd.memset(w_bdf[:], 0.0)

        # ---- x loads ----
        x_tiles = []
        xb_tiles = []
        for t in range(ntiles):
            x_t = x_pool.tile([128, FD], DT)
            x_tiles.append(x_t)
            xb_t = xb_pool.tile([128, FD], BF)
            xb_tiles.append(xb_t)
        # t0 tail right behind ident on sync (first compute needs it)
        nc.sync.dma_start(x_tiles[0][:, C0:FD], x_flat[0:128, C0:FD])
        nc.sync.dma_start(w_bdf[0:D, 0:D], w_dram)
        nc.sync.dma_start(w_bdf[D:128, D:128], w_dram)
        w_bd = const_pool.tile([128, 128], BF)
        nc.scalar.copy(w_bd[:], w_bdf[:])
        nc.scalar.dma_start(x_tiles[0][:, 0:C0], x_flat[0:128, 0:C0])
        for t in range(1, ntiles):
            b0 = t * 128
            nc.scalar.dma_start(x_tiles[t][:, 0:C0], x_flat[b0 : b0 + 128, 0:C0])
            nc.scalar.dma_start(x_tiles[t][:, C0:FD], x_flat[b0 : b0 + 128, C0:FD])

        # ---- phase A: PE + ACT chunk pipeline (both tiles) ----
        xw_tiles = []
        for t in range(ntiles):
            xw_t = xw_pool.tile([128, FD], DT)
            xw_tiles.append(xw_t)
        # bf16 converts are emitted lazily, right before the first chunk
        # that needs the given (tile, half) — ACT is in-order, so an early
        # convert whose x DMA hasn't landed would head-of-line-block the
        # tr/xw copies behind it
        cv_done = set()

        def ensure_cv(t, fp):
            # convert in [128, 640] pieces so a big convert can't
            # head-of-line-block the tr/xw copies on the ACT engine
            half = 1 if 2 * fp >= SPLIT_F else 0
            if (t, half) in cv_done:
                return
            cv_done.add((t, half))
            lo, hi = (C0, FD) if half else (0, C0)
            for c in range(lo, hi, 640):
                c1 = min(c + 640, hi)
                nc.scalar.copy(xb_tiles[t][:, c:c1], x_tiles[t][:, c:c1])

        # the first two chunks take the fp32 path: no dependency on the
        # bf16 converts or on w_bd, so the first output blocks are staged
        # ~6us earlier (PE is idle at this point; fp32 cost is irrelevant)
        FP32_CHUNKS = {(0, TAIL_FPS[0]), (0, TAIL_FPS[1])}

        for (t, fp) in CHUNK_ORDER:
            if t >= ntiles:
                continue
            xw_t = xw_tiles[t]
            if (t, fp) in FP32_CHUNKS:
                tr_ps = psum_tr.tile([128, 128], DT)
                nc.tensor.transpose(
                    tr_ps[:], x_tiles[t][:, fp * 128 : (fp + 1) * 128], ident[:]
                )
                tr_sb = tr_pool.tile([128, 128], DT)
                nc.scalar.copy(tr_sb[:], tr_ps[:])
                mm = psum_mm.tile([128, 128], DT, tag="mm")
                nc.tensor.matmul(mm[:], tr_sb[:], w_bdf[:], start=True, stop=True)
                nc.scalar.copy(xw_t[:, fp * 128 : (fp + 1) * 128], mm[:])
                continue
            ensure_cv(t, fp)
            xb_t = xb_tiles[t]
            tr_ps = psum_tr.tile([128, 128], BF)
            nc.tensor.transpose(
                tr_ps[:], xb_t[:, fp * 128 : (fp + 1) * 128], ident_bf[:]
            )
            tr_sb = tr_pool.tile([128, 128], BF)
            nc.scalar.copy(tr_sb[:], tr_ps[:])
            mm = psum_mm.tile([128, 128], DT, tag="mm")
            nc.tensor.matmul(mm[:], tr_sb[:], w_bd[:], start=True, stop=True)
            nc.scalar.copy(xw_t[:, fp * 128 : (fp + 1) * 128], mm[:])

        # ---- phase B: DVE muls + one output DMA per block, blocks
        # alternating between the sync and gpsimd DMA queues ----
        for k, (t, i) in enumerate(BLOCK_ORDER):
            if t >= ntiles:
                continue
            b0 = t * 128
            x_t, xw_t = x_tiles[t], xw_tiles[t]
            jn = F - 1 - i
            if jn <= SMALL_JN:
                pool = stage_as if k % 2 == 0 else stage_bs
            else:
                pool = stage_a if k % 2 == 0 else stage_b
            st = pool.tile([128, jn * D], DT)
            in0 = (
                xw_t[:, i * D : (i + 1) * D]
                .unsqueeze(1)
                .broadcast_to([128, jn, D])
            )
            in1 = x_t[:, (i + 1) * D : FD].rearrange("p (j d) -> p j d", d=D)
            nc.vector.tensor_mul(
                st[:].rearrange("p (j d) -> p j d", d=D), in0, in1
            )
            q = nc.sync if k % 2 == 0 else nc.gpsimd
            q.dma_start(
                out_dram[
                    b0 : b0 + 128,
                    BLOCK_OFF[i] * D : (BLOCK_OFF[i] + jn) * D,
                ],
                st[:],
            )

    nc.compile()
    return nc


def _get_nc(bs: int):
    if bs not in _CACHE:
        _CACHE[bs] = _build(bs)
    return _CACHE[bs]


def _run(inputs: np.ndarray, w: np.ndarray, trace: bool = False):
    inputs = np.ascontiguousarray(inputs, dtype=np.float32)
    w = np.ascontiguousarray(w, dtype=np.float32)
    assert inputs.shape == (B, F, D) and w.shape == (D, D)
    nc = _get_nc(BS)
    ident = np.eye(128, dtype=np.float32)
    in_maps = [
        {"x": inputs[c * BS : (c + 1) * BS], "w": w, "ident": ident}
        for c in range(NCORES)
    ]
    res = run_bass_kernel_spmd(nc, in_maps, list(range(NCORES)), trace=trace)
    out = np.concatenate([res.results[c]["out"] for c in range(NCORES)], axis=0)
    return out, res


def kernel(inputs: np.ndarray, w: np.ndarray) -> np.ndarray:
    out, _ = _run(inputs, w)
    return out


# revision 17
# speedup vs baseline: 1.0047x; 1.0047x over previous
"""BiLinearInteractionLayer (bilinear_type='all') Trainium2 Bass kernel.

Contract: kernel(inputs=[2048,40,64] f32, w=[64,64] f32) -> [2048, 49920] f32,
matching

    xw  = einsum('bfd,de->bfe', inputs, w)
    p   = xw[:, I, :] * inputs[:, J, :]   # (I, J) = triu_indices(40, k=1)
    out = p.reshape(B, -1)

Data-parallel over 8 NeuronCores: batch 2048 -> 8 x 256, W replicated.

v2 pipeline (per core, 2 x 128-row tiles):
  - x loads f32; ACT converts to bf16 for the PE path only
  - PE: bf16 transpose of each 2-field chunk (1-pass vs fp32's 2), then ONE
    bf16 matmul per chunk against a block-diag [[W,0],[0,W]] (f32 PSUM
    accumulate) -> xw chunk [128, 2*64] in one shot.  ~6x less PE time than
    the fp32 per-field scheme, so tile 1's xw is ready long before its muls.
  - pair muls (xw_i (x) v_j, f32, exact) split ~66/34 between DVE and Pool
    (gpsimd) so aggregate production rate stays above the ~410 GB/s DMA
    drain rate at all times.
  - ONE output DMA per 2-field chunk (40/core instead of 78): fewer
    semaphore updates (they ride DMA engine 79 and were the end-of-kernel
    straggler).
Only x and W are bf16-rounded inside the GEMM; the final elementwise product
is exact f32, so rel err ~1e-3 against the 2e-2 gate.
"""

import numpy as np
from contextlib import ExitStack

import concourse.bass as bass  # noqa: F401  (registers engines)
import concourse.bacc as bacc
import concourse.tile as tile
import concourse.mybir as mybir
from concourse.bass_utils import run_bass_kernel_spmd

B = 2048
F = 40
D = 64
NCORES = 8
BS = B // NCORES                   # 256 rows per core
PAIRS = F * (F - 1) // 2           # 780
OUT_W = PAIRS * D                  # 49920
FD = F * D                         # 2560
DT = mybir.dt.float32
BF = mybir.dt.bfloat16

BLOCK_LEN = [F - 1 - i for i in range(F - 1)]
BLOCK_OFF = np.concatenate([[0], np.cumsum(BLOCK_LEN)[:-1]]).tolist()

# chunk fp covers fields (2fp, 2fp+1); tail chunks (fields >= 30) first for
# tile 0 so the output stream starts as soon as the tail x DMA lands
SPLIT_F = 30
C0 = SPLIT_F * D                    # tail split column (f32 x)
TAIL_FPS = list(range(SPLIT_F // 2, F // 2))   # 15..19
HEAD_FPS = list(range(SPLIT_F // 2))           # 0..14

# phase-A (PE/ACT) chunk order, tile-sequential (interleaving the tiles
# measurably slowed both DVE and the HBM drain): t0 tail chunks first (ready
# as soon as the tail x DMA lands), then t0 heads, t1 heads, t1 tails
CHUNK_ORDER = (
    [(0, fp) for fp in TAIL_FPS]
    + [(0, fp) for fp in HEAD_FPS]
    + [(1, fp) for fp in HEAD_FPS]
    + [(1, fp) for fp in TAIL_FPS]
)

# phase-B (DVE mul + per-block DMA) production order: a small warmup burst
# from t0's tail fields (they only need the tail x chunk), then big blocks
# descending (production ~490 GB/s beats the ~418 GB/s drain, banking
# backlog in the stage ring), then t1's small tail blocks which drain out
# of the banked backlog.
BLOCK_ORDER = (
    [(0, i) for i in range(SPLIT_F, F - 1)]
    + [(0, i) for i in range(SPLIT_F)]
    + [(1, i) for i in range(SPLIT_F)]
    + [(1, i) for i in range(SPLIT_F, F - 1)]
)

_CACHE = {}


def _build(bs: int):
    assert bs % 128 == 0
    ntiles = bs // 128
    nc = bacc.Bacc("TRN2", target_bir_lowering=False, debug=False)

    x_dram = nc.dram_tensor("x", [bs, F, D], DT, kind="ExternalInput").ap()
    w_dram = nc.dram_tensor("w", [D, D], DT, kind="ExternalInput").ap()
    id_dram = nc.dram_tensor("ident", [128, 128], DT, kind="ExternalInput").ap()
    out_dram = nc.dram_tensor("out", [bs, OUT_W], DT, kind="ExternalOutput").ap()

    x_flat = x_dram.rearrange("b f d -> b (f d)")

    with tile.TileContext(nc) as tc, ExitStack() as ctx:
        const_pool = ctx.enter_context(tc.tile_pool(name="const", bufs=1))
        x_pool = ctx.enter_context(tc.tile_pool(name="x", bufs=2))
        xb_pool = ctx.enter_context(tc.tile_pool(name="xb", bufs=2))
        xw_pool = ctx.enter_context(tc.tile_pool(name="xw", bufs=2))
        tr_pool = ctx.enter_context(tc.tile_pool(name="tr", bufs=3))
        # one stage ring per output DMA queue; alternating blocks between two
        # queues hides each queue's slot-free -> mul -> issue chain latency
        # behind the other queue's drains.  Small blocks get their own deeper
        # rings (slots are max-size, so mixing them with jn=39 blocks wastes
        # SBUF and leaves the tail with too little elasticity).
        stage_a = ctx.enter_context(tc.tile_pool(name="stage_a", bufs=6))
        stage_b = ctx.enter_context(tc.tile_pool(name="stage_b", bufs=6))
        psum_tr = ctx.enter_context(tc.tile_pool(name="psum_tr", bufs=3, space="PSUM"))
        psum_mm = ctx.enter_context(tc.tile_pool(name="psum_mm", bufs=4, space="PSUM"))

        # ---- constants (sync queue: starts clean, lands earliest) ----
        ident = const_pool.tile([128, 128], DT)
        nc.sync.dma_start(ident[:], id_dram)
        ident_bf = const_pool.tile([128, 128], BF)
        nc.scalar.copy(ident_bf[:], ident[:])

        # f32 block-diag [[W,0],[0,W]] assembled by DMA into a zeroed tile,
        # then one ACT convert to bf16
        w_bdf = const_pool.tile([128, 128], DT)
        nc.gpsimd.memset(w_bdf[:], 0.0)

        # ---- x loads ----
        x_tiles = []
        xb_tiles = []
        for t in range(ntiles):
            x_t = x_pool.tile([128, FD], DT)
            x_tiles.append(x_t)
            xb_t = xb_pool.tile([128, FD], BF)
            xb_tiles.append(xb_t)
        # t0 tail right behind ident on sync (first compute needs it)
        nc.sync.dma_start(x_tiles[0][:, C0:FD], x_flat[0:128, C0:FD])
        nc.sync.dma_start(w_bdf[0:D, 0:D], w_dram)
        nc.sync.dma_start(w_bdf[D:128, D:128], w_dram)
        w_bd = const_pool.tile([128, 128], BF)
        nc.scalar.copy(w_bd[:], w_bdf[:])
        nc.scalar.dma_start(x_tiles[0][:, 0:C0], x_flat[0:128, 0:C0])
        for t in range(1, ntiles):
            b0 = t * 128
            nc.scalar.dma_start(x_tiles[t][:, 0:C0], x_flat[b0 : b0 + 128, 0:C0])
            nc.scalar.dma_start(x_tiles[t][:, C0:FD], x_flat[b0 : b0 + 128, C0:FD])

        # ---- phase A: PE + ACT chunk pipeline (both tiles) ----
        xw_tiles = []
        for t in range(ntiles):
            xw_t = xw_pool.tile([128, FD], DT)
            xw_tiles.append(xw_t)
        # bf16 converts are emitted lazily, right before the first chunk
        # that needs the given (tile, half) — ACT is in-order, so an early
        # convert whose x DMA hasn't landed would head-of-line-block the
        # tr/xw copies behind it
        cv_done = set()

        def ensure_cv(t, fp):
            # convert in [128, 640] pieces so a big convert can't
            # head-of-line-block the tr/xw copies on the ACT engine
            half = 1 if 2 * fp >= SPLIT_F else 0
            if (t, half) in cv_done:
                return
            cv_done.add((t, half))
            lo, hi = (C0, FD) if half else (0, C0)
            for c in range(lo, hi, 640):
                c1 = min(c + 640, hi)
                nc.scalar.copy(xb_tiles[t][:, c:c1], x_tiles[t][:, c:c1])

        # the first two chunks take the fp32 path: no dependency on the
        # bf16 converts or on w_bd, so the first output blocks are staged
        # ~6us earlier (PE is idle at this point; fp32 cost is irrelevant)
        FP32_CHUNKS = {(0, TAIL_FPS[0]), (0, TAIL_FPS[1])}

        for (t, fp) in CHUNK_ORDER:
            if t >= ntiles:
                continue
            xw_t = xw_tiles[t]
            if (t, fp) in FP32_CHUNKS:
                tr_ps = psum_tr.tile([128, 128], DT)
                nc.tensor.transpose(
                    tr_ps[:], x_tiles[t][:, fp * 128 : (fp + 1) * 128], ident[:]
                )
                tr_sb = tr_pool.tile([128, 128], DT)
                nc.scalar.copy(tr_sb[:], tr_ps[:])
                mm = psum_mm.tile([128, 128], DT, tag="mm")
                nc.tensor.matmul(mm[:], tr_sb[:], w_bdf[:], start=True, stop=True)
                nc.scalar.copy(xw_t[:, fp * 128 : (fp + 1) * 128], mm[:])
                continue
            ensure_cv(t, fp)
            xb_t = xb_tiles[t]
            tr_ps = psum_tr.tile([128, 128], BF)
            nc.tensor.transpose(
                tr_ps[:], xb_t[:, fp * 128 : (fp + 1) * 128], ident_bf[:]
            )
            tr_sb = tr_pool.tile([128, 128], BF)
            nc.scalar.copy(tr_sb[:], tr_ps[:])
            mm = psum_mm.tile([128, 128], DT, tag="mm")
            nc.tensor.matmul(mm[:], tr_sb[:], w_bd[:], start=True, stop=True)
            nc.scalar.copy(xw_t[:, fp * 128 : (fp + 1) * 128], mm[:])

        # ---- phase B: DVE muls + one output DMA per block, blocks
        # alternating between the sync and gpsimd DMA queues ----
        for k, (t, i) in enumerate(BLOCK_ORDER):
            if t >= ntiles:
                continue
            b0 = t * 128
            x_t, xw_t = x_tiles[t], xw_tiles[t]
            jn = F - 1 - i
            pool = stage_a if k % 2 == 0 else stage_b
            st = pool.tile([128, jn * D], DT)
            in0 = (
                xw_t[:, i * D : (i + 1) * D]
                .unsqueeze(1)
                .broadcast_to([128, jn, D])
            )
            in1 = x_t[:, (i + 1) * D : FD].rearrange("p (j d) -> p j d", d=D)
            nc.vector.tensor_mul(
                st[:].rearrange("p (j d) -> p j d", d=D), in0, in1
            )
            q = nc.sync if k % 2 == 0 else nc.gpsimd
            q.dma_start(
                out_dram[
                    b0 : b0 + 128,
                    BLOCK_OFF[i] * D : (BLOCK_OFF[i] + jn) * D,
                ],
                st[:],
            )

    nc.compile()
    return nc


def _get_nc(bs: int):
    if bs not in _CACHE:
        _CACHE[bs] = _build(bs)
    return _CACHE[bs]


def _run(inputs: np.ndarray, w: np.ndarray, trace: bool = False):
    inputs = np.ascontiguousarray(inputs, dtype=np.float32)
    w = np.ascontiguousarray(w, dtype=np.float32)
    assert inputs.shape == (B, F, D) and w.shape == (D, D)
    nc = _get_nc(BS)
    ident = np.eye(128, dtype=np.float32)
    in_maps = [
        {"x": inputs[c * BS : (c + 1) * BS], "w": w, "ident": ident}
        for c in range(NCORES)
    ]
    res = run_bass_kernel_spmd(nc, in_maps, list(range(NCORES)), trace=trace)
    out = np.concatenate([res.results[c]["out"] for c in range(NCORES)], axis=0)
    return out, res


def kernel(inputs: np.ndarray, w: np.ndarray) -> np.ndarray:
    out, _ = _run(inputs, w)
    return out


# revision 21
# speedup vs baseline: 1.0110x; 1.0063x over previous
"""BiLinearInteractionLayer (bilinear_type='all') Trainium2 Bass kernel.

Contract: kernel(inputs=[2048,40,64] f32, w=[64,64] f32) -> [2048, 49920] f32,
matching

    xw  = einsum('bfd,de->bfe', inputs, w)
    p   = xw[:, I, :] * inputs[:, J, :]   # (I, J) = triu_indices(40, k=1)
    out = p.reshape(B, -1)

Data-parallel over 8 NeuronCores: batch 2048 -> 8 x 256, W replicated.

v8 pipeline (per core, 2 x 128-row tiles):
  - bf16 identity and bf16 block-diag [[W,0],[0,W]] are built on the HOST
    and DMAd as tiny (32 KB) constants: the on-device W assembly chain
    (memset + 2 DMAs + convert) was on the first-output critical path
    because the DMA system crawls for its first ~15 us.
  - PE path: ACT converts x to bf16, PE does a 1-pass bf16 transpose per
    2-field chunk, then ONE bf16 matmul per chunk against the block-diag W
    (f32 PSUM accumulate) -> xw chunk [128, 128].  ~6x less PE time than
    fp32, so both tiles' xw is ready ~30 us in and the DVE mul stream never
    waits on PE.
  - pair muls xw_i (x) v_j stay exact f32 on DVE (the only bf16 rounding is
    inside the GEMM: rel err ~2.7e-3 vs the 2e-2 gate).
  - per-block output DMAs alternate between the sync (HWDGE) and gpsimd
    (SWDGE) queues with a 6-deep stage ring each: the second queue hides
    each queue's slot-free -> mul -> issue chain latency.
  - warmup: block order starts at i=38 (needs only 32 KB of x + the two
    constants), so the first output DMA issues as early as the crawling
    early-DMA window allows; the x tail is loaded in two pieces to match.
  - the last 6 blocks get dedicated (non-ring) stage tiles so the tail is
    pure DMA drain instead of a serialized slot-free chain.
"""

import numpy as np
import ml_dtypes
from contextlib import ExitStack

import concourse.bass as bass  # noqa: F401  (registers engines)
import concourse.bacc as bacc
import concourse.tile as tile
import concourse.mybir as mybir
from concourse.bass_utils import run_bass_kernel_spmd

B = 2048
F = 40
D = 64
NCORES = 8
BS = B // NCORES                   # 256 rows per core
PAIRS = F * (F - 1) // 2           # 780
OUT_W = PAIRS * D                  # 49920
FD = F * D                         # 2560
DT = mybir.dt.float32
BF = mybir.dt.bfloat16
BF_NP = ml_dtypes.bfloat16

BLOCK_LEN = [F - 1 - i for i in range(F - 1)]
BLOCK_OFF = np.concatenate([[0], np.cumsum(BLOCK_LEN)[:-1]]).tolist()

SPLIT_F = 30
C0 = SPLIT_F * D                   # tail split column (f32 x)
C1 = 36 * D                        # warmup split: fields 36..40
TAIL_FPS_T0 = [19, 18, 17, 16, 15]   # fp19 first: block 38 needs only it
TAIL_FPS_T1 = [15, 16, 17, 18, 19]
HEAD_FPS = list(range(SPLIT_F // 2))           # 0..14

CHUNK_ORDER = (
    [(0, fp) for fp in TAIL_FPS_T0]
    + [(0, fp) for fp in HEAD_FPS]
    + [(1, fp) for fp in HEAD_FPS]
    + [(1, fp) for fp in TAIL_FPS_T1]
)

# production order: minimal-dependency warmup (i=38 downward), then big
# blocks descending for both tiles, then t1's tail blocks
BLOCK_ORDER = (
    [(0, i) for i in range(F - 2, SPLIT_F - 1, -1)]
    + [(0, i) for i in range(SPLIT_F)]
    + [(1, i) for i in range(SPLIT_F)]
    + [(1, i) for i in range(SPLIT_F, F - 1)]
)
N_END = 6   # last N blocks use dedicated stage tiles (no ring)

# x->bf16 convert units (columns), finest where the warmup needs them
CV_UNITS_T0 = [(C1, FD), (C0, C1), (0, 640), (640, 1280), (1280, 1920)]
CV_UNITS_T1 = [(0, 640), (640, 1280), (1280, 1920), (C0, C1), (C1, FD)]

_CACHE = {}


def _build(bs: int):
    assert bs % 128 == 0
    ntiles = bs // 128
    nc = bacc.Bacc("TRN2", target_bir_lowering=False, debug=False)

    x_dram = nc.dram_tensor("x", [bs, F, D], DT, kind="ExternalInput").ap()
    wbd_dram = nc.dram_tensor("wbd", [128, 128], BF, kind="ExternalInput").ap()
    id_dram = nc.dram_tensor("identb", [128, 128], BF, kind="ExternalInput").ap()
    out_dram = nc.dram_tensor("out", [bs, OUT_W], DT, kind="ExternalOutput").ap()

    x_flat = x_dram.rearrange("b f d -> b (f d)")

    with tile.TileContext(nc) as tc, ExitStack() as ctx:
        const_pool = ctx.enter_context(tc.tile_pool(name="const", bufs=1))
        x_pool = ctx.enter_context(tc.tile_pool(name="x", bufs=2))
        xb_pool = ctx.enter_context(tc.tile_pool(name="xb", bufs=2))
        xw_pool = ctx.enter_context(tc.tile_pool(name="xw", bufs=2))
        tr_pool = ctx.enter_context(tc.tile_pool(name="tr", bufs=3))
        stage_a = ctx.enter_context(tc.tile_pool(name="stage_a", bufs=5))
        stage_b = ctx.enter_context(tc.tile_pool(name="stage_b", bufs=5))
        stage_end = ctx.enter_context(tc.tile_pool(name="stage_end", bufs=N_END))
        psum_tr = ctx.enter_context(tc.tile_pool(name="psum_tr", bufs=3, space="PSUM"))
        psum_mm = ctx.enter_context(tc.tile_pool(name="psum_mm", bufs=4, space="PSUM"))

        # ---- constants: two 32 KB DMAs, nothing else ----
        ident_bf = const_pool.tile([128, 128], BF)
        nc.sync.dma_start(ident_bf[:], id_dram)
        w_bd = const_pool.tile([128, 128], BF)
        nc.sync.dma_start(w_bd[:], wbd_dram)

        # ---- x loads ----
        x_tiles = []
        xb_tiles = []
        for t in range(ntiles):
            x_tiles.append(x_pool.tile([128, FD], DT, name=f"x{t}"))
            xb_tiles.append(xb_pool.tile([128, FD], BF, name=f"xb{t}"))
        # t0 tail in two pieces on sync; warmup piece (fields 36..40) first
        nc.sync.dma_start(x_tiles[0][:, C1:FD], x_flat[0:128, C1:FD])
        nc.sync.dma_start(x_tiles[0][:, C0:C1], x_flat[0:128, C0:C1])
        nc.scalar.dma_start(x_tiles[0][:, 0:C0], x_flat[0:128, 0:C0])
        for t in range(1, ntiles):
            b0 = t * 128
            nc.scalar.dma_start(x_tiles[t][:, 0:C0], x_flat[b0 : b0 + 128, 0:C0])
            nc.scalar.dma_start(x_tiles[t][:, C0:FD], x_flat[b0 : b0 + 128, C0:FD])

        # ---- phase A: PE + ACT chunk pipeline (both tiles) ----
        xw_tiles = []
        for t in range(ntiles):
            xw_tiles.append(xw_pool.tile([128, FD], DT, name=f"xw{t}"))

        cv_done = set()

        def ensure_cv(t, fp):
            units = CV_UNITS_T0 if t == 0 else CV_UNITS_T1
            lo_need, hi_need = fp * 128, (fp + 1) * 128
            for u, (lo, hi) in enumerate(units):
                if lo < hi_need and hi > lo_need and (t, u) not in cv_done:
                    cv_done.add((t, u))
                    nc.scalar.copy(xb_tiles[t][:, lo:hi], x_tiles[t][:, lo:hi])

        for (t, fp) in CHUNK_ORDER:
            if t >= ntiles:
                continue
            ensure_cv(t, fp)
            xb_t, xw_t = xb_tiles[t], xw_tiles[t]
            tr_ps = psum_tr.tile([128, 128], BF)
            nc.tensor.transpose(
                tr_ps[:], xb_t[:, fp * 128 : (fp + 1) * 128], ident_bf[:]
            )
            tr_sb = tr_pool.tile([128, 128], BF)
            nc.scalar.copy(tr_sb[:], tr_ps[:])
            mm = psum_mm.tile([128, 128], DT, tag="mm")
            nc.tensor.matmul(mm[:], tr_sb[:], w_bd[:], start=True, stop=True)
            nc.scalar.copy(xw_t[:, fp * 128 : (fp + 1) * 128], mm[:])

        # ---- phase B: DVE muls + per-block DMAs on alternating queues ----
        n_blocks = len(BLOCK_ORDER)
        for k, (t, i) in enumerate(BLOCK_ORDER):
            if t >= ntiles:
                continue
            b0 = t * 128
            x_t, xw_t = x_tiles[t], xw_tiles[t]
            jn = F - 1 - i
            if k >= n_blocks - N_END:
                pool = stage_end
            else:
                pool = stage_a if k % 2 == 0 else stage_b
            st = pool.tile([128, jn * D], DT)
            in0 = (
                xw_t[:, i * D : (i + 1) * D]
                .unsqueeze(1)
                .broadcast_to([128, jn, D])
            )
            in1 = x_t[:, (i + 1) * D : FD].rearrange("p (j d) -> p j d", d=D)
            nc.vector.tensor_mul(
                st[:].rearrange("p (j d) -> p j d", d=D), in0, in1
            )
            q = nc.sync if k % 2 == 0 else nc.gpsimd
            q.dma_start(
                out_dram[
                    b0 : b0 + 128,
                    BLOCK_OFF[i] * D : (BLOCK_OFF[i] + jn) * D,
                ],
                st[:],
            )

    nc.compile()
    return nc


def _get_nc(bs: int):
    if bs not in _CACHE:
        _CACHE[bs] = _build(bs)
    return _CACHE[bs]


def _run(inputs: np.ndarray, w: np.ndarray, trace: bool = False):
    inputs = np.ascontiguousarray(inputs, dtype=np.float32)
    w = np.ascontiguousarray(w, dtype=np.float32)
    assert inputs.shape == (B, F, D) and w.shape == (D, D)
    nc = _get_nc(BS)
    identb = np.eye(128, dtype=BF_NP)
    wbd = np.zeros((128, 128), dtype=BF_NP)
    wbd[0:D, 0:D] = w.astype(BF_NP)
    wbd[D:128, D:128] = w.astype(BF_NP)
    in_maps = [
        {"x": inputs[c * BS : (c + 1) * BS], "wbd": wbd, "identb": identb}
        for c in range(NCORES)
    ]
    res = run_bass_kernel_spmd(nc, in_maps, list(range(NCORES)), trace=trace)
    out = np.concatenate([res.results[c]["out"] for c in range(NCORES)], axis=0)
    return out, res


def kernel(inputs: np.ndarray, w: np.ndarray) -> np.ndarray:
    out, _ = _run(inputs, w)
    return out


# revision 22
# speedup vs baseline: 1.1911x; 1.1781x over previous
"""BiLinearInteractionLayer (bilinear_type='all') Trainium2 Bass kernel.

Contract: kernel(inputs=[2048,40,64] f32, w=[64,64] f32) -> [2048, 49920] f32,
matching

    xw  = einsum('bfd,de->bfe', inputs, w)
    p   = xw[:, I, :] * inputs[:, J, :]   # (I, J) = triu_indices(40, k=1)
    out = p.reshape(B, -1)

Data-parallel over 8 NeuronCores: batch 2048 -> 8 x 256, W replicated.

v9 pipeline (per core, 2 x 128-row tiles):
  - bf16 block-diag [[W,0],[0,W]] built on the HOST (32 KB constant); the
    f32 identity is the only other constant.  The early DMA window crawls,
    so the first compute depends on as few bytes as possible.
  - PE path per 2-field chunk: fp32 transpose of the x chunk, ACT copies
    PSUM -> SBUF casting to bf16, then ONE bf16 matmul against the
    block-diag W (f32 PSUM) -> xw chunk [128, 128].  No separate x->bf16
    convert pass; PE stays far ahead of the DVE mul stream.
  - pair muls xw_i (x) v_j are exact f32 on DVE (only the GEMM inputs are
    bf16-rounded: rel err ~2.7e-3 vs the 2e-2 gate).
  - ALL output blocks go out on the single sync HWDGE queue: splitting
    across two queues costs ~8% per-DMA-engine efficiency (41-43 vs 38.7
    ns/KB measured).
  - warmup: block order starts at i=38 (needs only 32 KB of x + constants),
    x tail loaded in two pieces to match; then big blocks descending, t0
    then t1; the 9-deep stage ring banks several MB of backlog so the DMA
    queue never starves mid-run.
  - the last 9 blocks get dedicated (non-ring) stage tiles so the tail is
    pure queued drain instead of a serialized slot-free -> mul -> issue
    chain.
"""

import numpy as np
import ml_dtypes
from contextlib import ExitStack

import concourse.bass as bass  # noqa: F401  (registers engines)
import concourse.bacc as bacc
import concourse.tile as tile
import concourse.mybir as mybir
from concourse.bass_utils import run_bass_kernel_spmd

B = 2048
F = 40
D = 64
NCORES = 8
BS = B // NCORES                   # 256 rows per core
PAIRS = F * (F - 1) // 2           # 780
OUT_W = PAIRS * D                  # 49920
FD = F * D                         # 2560
DT = mybir.dt.float32
BF = mybir.dt.bfloat16
BF_NP = ml_dtypes.bfloat16

BLOCK_LEN = [F - 1 - i for i in range(F - 1)]
BLOCK_OFF = np.concatenate([[0], np.cumsum(BLOCK_LEN)[:-1]]).tolist()

SPLIT_F = 30
C0 = SPLIT_F * D                   # tail split column
C1 = 36 * D                        # warmup split: fields 36..40
TAIL_FPS_T0 = [19, 18, 17, 16, 15]   # fp19 first: block 38 needs only it
TAIL_FPS_T1 = [15, 16, 17, 18, 19]
HEAD_FPS = list(range(SPLIT_F // 2))           # 0..14

CHUNK_ORDER = (
    [(0, fp) for fp in TAIL_FPS_T0]
    + [(0, fp) for fp in HEAD_FPS]
    + [(1, fp) for fp in HEAD_FPS]
    + [(1, fp) for fp in TAIL_FPS_T1]
)

# production order: minimal-dependency warmup (i=38 downward), then big
# blocks descending for both tiles, then t1's tail blocks
BLOCK_ORDER = (
    [(0, i) for i in range(F - 2, SPLIT_F - 1, -1)]
    + [(0, i) for i in range(SPLIT_F)]
    + [(1, i) for i in range(SPLIT_F)]
    + [(1, i) for i in range(SPLIT_F, F - 1)]
)
N_END = 9   # last N blocks use dedicated stage tiles (no ring)

_CACHE = {}


def _build(bs: int):
    assert bs % 128 == 0
    ntiles = bs // 128
    nc = bacc.Bacc("TRN2", target_bir_lowering=False, debug=False)

    x_dram = nc.dram_tensor("x", [bs, F, D], DT, kind="ExternalInput").ap()
    wbd_dram = nc.dram_tensor("wbd", [128, 128], BF, kind="ExternalInput").ap()
    id_dram = nc.dram_tensor("ident", [128, 128], DT, kind="ExternalInput").ap()
    out_dram = nc.dram_tensor("out", [bs, OUT_W], DT, kind="ExternalOutput").ap()

    x_flat = x_dram.rearrange("b f d -> b (f d)")

    with tile.TileContext(nc) as tc, ExitStack() as ctx:
        const_pool = ctx.enter_context(tc.tile_pool(name="const", bufs=1))
        x_pool = ctx.enter_context(tc.tile_pool(name="x", bufs=2))
        xw_pool = ctx.enter_context(tc.tile_pool(name="xw", bufs=2))
        tr_pool = ctx.enter_context(tc.tile_pool(name="tr", bufs=3))
        stage_a = ctx.enter_context(tc.tile_pool(name="stage_a", bufs=9))
        stage_end = ctx.enter_context(tc.tile_pool(name="stage_end", bufs=N_END))
        psum_tr = ctx.enter_context(tc.tile_pool(name="psum_tr", bufs=3, space="PSUM"))
        psum_mm = ctx.enter_context(tc.tile_pool(name="psum_mm", bufs=4, space="PSUM"))

        # ---- constants ----
        w_bd = const_pool.tile([128, 128], BF)
        nc.sync.dma_start(w_bd[:], wbd_dram)
        ident = const_pool.tile([128, 128], DT)
        nc.sync.dma_start(ident[:], id_dram)

        # ---- x loads ----
        x_tiles = []
        for t in range(ntiles):
            x_tiles.append(x_pool.tile([128, FD], DT, name=f"x{t}"))
        # t0 tail in two pieces on sync; warmup piece (fields 36..40) first
        nc.sync.dma_start(x_tiles[0][:, C1:FD], x_flat[0:128, C1:FD])
        nc.sync.dma_start(x_tiles[0][:, C0:C1], x_flat[0:128, C0:C1])
        nc.scalar.dma_start(x_tiles[0][:, 0:C0], x_flat[0:128, 0:C0])
        for t in range(1, ntiles):
            b0 = t * 128
            nc.scalar.dma_start(x_tiles[t][:, 0:C0], x_flat[b0 : b0 + 128, 0:C0])
            nc.scalar.dma_start(x_tiles[t][:, C0:FD], x_flat[b0 : b0 + 128, C0:FD])

        # ---- phase A: PE + ACT chunk pipeline (both tiles) ----
        xw_tiles = []
        for t in range(ntiles):
            xw_tiles.append(xw_pool.tile([128, FD], DT, name=f"xw{t}"))

        for (t, fp) in CHUNK_ORDER:
            if t >= ntiles:
                continue
            x_t, xw_t = x_tiles[t], xw_tiles[t]
            tr_ps = psum_tr.tile([128, 128], DT)
            nc.tensor.transpose(
                tr_ps[:], x_t[:, fp * 128 : (fp + 1) * 128], ident[:]
            )
            tr_sb = tr_pool.tile([128, 128], BF)   # cast f32->bf16 in the copy
            nc.scalar.copy(tr_sb[:], tr_ps[:])
            mm = psum_mm.tile([128, 128], DT, tag="mm")
            nc.tensor.matmul(mm[:], tr_sb[:], w_bd[:], start=True, stop=True)
            nc.scalar.copy(xw_t[:, fp * 128 : (fp + 1) * 128], mm[:])

        # ---- phase B: DVE muls + per-block DMAs on the sync queue ----
        n_blocks = len(BLOCK_ORDER)
        for k, (t, i) in enumerate(BLOCK_ORDER):
            if t >= ntiles:
                continue
            b0 = t * 128
            x_t, xw_t = x_tiles[t], xw_tiles[t]
            jn = F - 1 - i
            pool = stage_end if k >= n_blocks - N_END else stage_a
            st = pool.tile([128, jn * D], DT)
            in0 = (
                xw_t[:, i * D : (i + 1) * D]
                .unsqueeze(1)
                .broadcast_to([128, jn, D])
            )
            in1 = x_t[:, (i + 1) * D : FD].rearrange("p (j d) -> p j d", d=D)
            nc.vector.tensor_mul(
                st[:].rearrange("p (j d) -> p j d", d=D), in0, in1
            )
            nc.sync.dma_start(
                out_dram[
                    b0 : b0 + 128,
                    BLOCK_OFF[i] * D : (BLOCK_OFF[i] + jn) * D,
                ],
                st[:],
            )

    nc.compile()
    return nc


def _get_nc(bs: int):
    if bs not in _CACHE:
        _CACHE[bs] = _build(bs)
    return _CACHE[bs]


def _run(inputs: np.ndarray, w: np.ndarray, trace: bool = False):
    inputs = np.ascontiguousarray(inputs, dtype=np.float32)
    w = np.ascontiguousarray(w, dtype=np.float32)
    assert inputs.shape == (B, F, D) and w.shape == (D, D)
    nc = _get_nc(BS)
    ident = np.eye(128, dtype=np.float32)
    wbd = np.zeros((128, 128), dtype=BF_NP)
    wbd[0:D, 0:D] = w.astype(BF_NP)
    wbd[D:128, D:128] = w.astype(BF_NP)
    in_maps = [
        {"x": inputs[c * BS : (c + 1) * BS], "wbd": wbd, "ident": ident}
        for c in range(NCORES)
    ]
    res = run_bass_kernel_spmd(nc, in_maps, list(range(NCORES)), trace=trace)
    out = np.concatenate([res.results[c]["out"] for c in range(NCORES)], axis=0)
    return out, res


def kernel(inputs: np.ndarray, w: np.ndarray) -> np.ndarray:
    out, _ = _run(inputs, w)
    return out


# revision 25
# speedup vs baseline: 1.2011x; 1.0084x over previous
"""BiLinearInteractionLayer (bilinear_type='all') Trainium2 Bass kernel.

Contract: kernel(inputs=[2048,40,64] f32, w=[64,64] f32) -> [2048, 49920] f32,
matching

    xw  = einsum('bfd,de->bfe', inputs, w)
    p   = xw[:, I, :] * inputs[:, J, :]   # (I, J) = triu_indices(40, k=1)
    out = p.reshape(B, -1)

Data-parallel over 8 NeuronCores: batch 2048 -> 8 x 256, W replicated.

v9 pipeline (per core, 2 x 128-row tiles):
  - bf16 block-diag [[W,0],[0,W]] built on the HOST (32 KB constant); the
    f32 identity is the only other constant.  The early DMA window crawls,
    so the first compute depends on as few bytes as possible.
  - PE path per 2-field chunk: fp32 transpose of the x chunk, ACT copies
    PSUM -> SBUF casting to bf16, then ONE bf16 matmul against the
    block-diag W (f32 PSUM) -> xw chunk [128, 128].  No separate x->bf16
    convert pass; PE stays far ahead of the DVE mul stream.
  - pair muls xw_i (x) v_j are exact f32 on DVE (only the GEMM inputs are
    bf16-rounded: rel err ~2.7e-3 vs the 2e-2 gate).
  - ALL output blocks go out on the single sync HWDGE queue: splitting
    across two queues costs ~8% per-DMA-engine efficiency (41-43 vs 38.7
    ns/KB measured).
  - warmup: block order starts at i=38 (needs only 32 KB of x + constants),
    x tail loaded in two pieces to match; then big blocks descending, t0
    then t1; the 9-deep stage ring banks several MB of backlog so the DMA
    queue never starves mid-run.
  - the last 9 blocks get dedicated (non-ring) stage tiles so the tail is
    pure queued drain instead of a serialized slot-free -> mul -> issue
    chain.
"""

import numpy as np
import ml_dtypes
from contextlib import ExitStack

import concourse.bass as bass  # noqa: F401  (registers engines)
import concourse.bacc as bacc
import concourse.tile as tile
import concourse.mybir as mybir
from concourse.bass_utils import run_bass_kernel_spmd

B = 2048
F = 40
D = 64
NCORES = 8
BS = B // NCORES                   # 256 rows per core
PAIRS = F * (F - 1) // 2           # 780
OUT_W = PAIRS * D                  # 49920
FD = F * D                         # 2560
DT = mybir.dt.float32
BF = mybir.dt.bfloat16
BF_NP = ml_dtypes.bfloat16

BLOCK_LEN = [F - 1 - i for i in range(F - 1)]
BLOCK_OFF = np.concatenate([[0], np.cumsum(BLOCK_LEN)[:-1]]).tolist()

SPLIT_F = 30
C0 = SPLIT_F * D                   # tail split column
C1 = 36 * D                        # warmup split boundary
C2 = 38 * D                        # first warmup piece: fields 38..40
TAIL_FPS_T0 = [19, 18, 17, 16, 15]   # fp19 first: block 38 needs only it
TAIL_FPS_T1 = [15, 16, 17, 18, 19]
HEAD_FPS = list(range(SPLIT_F // 2))           # 0..14

CHUNK_ORDER = (
    [(0, fp) for fp in TAIL_FPS_T0]
    + [(0, fp) for fp in HEAD_FPS]
    + [(1, fp) for fp in HEAD_FPS]
    + [(1, fp) for fp in TAIL_FPS_T1]
)

# production order: minimal-dependency warmup (i=38 downward), then big
# blocks descending for both tiles, then t1's tail blocks
BLOCK_ORDER = (
    [(0, i) for i in range(F - 2, SPLIT_F - 1, -1)]
    + [(0, i) for i in range(SPLIT_F)]
    + [(1, i) for i in range(SPLIT_F)]
    + [(1, i) for i in range(SPLIT_F, F - 1)]
)
N_END = 9   # last N blocks use dedicated stage tiles (no ring)

_CACHE = {}


def _build(bs: int):
    assert bs % 128 == 0
    ntiles = bs // 128
    nc = bacc.Bacc("TRN2", target_bir_lowering=False, debug=False)

    x_dram = nc.dram_tensor("x", [bs, F, D], DT, kind="ExternalInput").ap()
    wbd_dram = nc.dram_tensor("wbd", [128, 128], BF, kind="ExternalInput").ap()
    id_dram = nc.dram_tensor("ident", [128, 128], DT, kind="ExternalInput").ap()
    out_dram = nc.dram_tensor("out", [bs, OUT_W], DT, kind="ExternalOutput").ap()

    x_flat = x_dram.rearrange("b f d -> b (f d)")

    with tile.TileContext(nc) as tc, ExitStack() as ctx:
        const_pool = ctx.enter_context(tc.tile_pool(name="const", bufs=1))
        x_pool = ctx.enter_context(tc.tile_pool(name="x", bufs=2))
        xw_pool = ctx.enter_context(tc.tile_pool(name="xw", bufs=2))
        tr_pool = ctx.enter_context(tc.tile_pool(name="tr", bufs=3))
        stage_a = ctx.enter_context(tc.tile_pool(name="stage_a", bufs=10))
        stage_end = ctx.enter_context(tc.tile_pool(name="stage_end", bufs=1))
        psum_tr = ctx.enter_context(tc.tile_pool(name="psum_tr", bufs=3, space="PSUM"))
        psum_mm = ctx.enter_context(tc.tile_pool(name="psum_mm", bufs=4, space="PSUM"))

        # ---- constants ----
        ident = const_pool.tile([128, 128], DT)
        nc.sync.dma_start(ident[:], id_dram)
        w_bd = const_pool.tile([128, 128], BF)
        nc.sync.dma_start(w_bd[:], wbd_dram)

        # ---- x loads ----
        x_tiles = []
        for t in range(ntiles):
            x_tiles.append(x_pool.tile([128, FD], DT, name=f"x{t}"))
        # t0 tail in pieces on sync; smallest warmup piece (fields 38..40)
        # first so block 38's chain starts as early as the DMA crawl allows
        nc.sync.dma_start(x_tiles[0][:, C2:FD], x_flat[0:128, C2:FD])
        nc.sync.dma_start(x_tiles[0][:, C1:C2], x_flat[0:128, C1:C2])
        nc.sync.dma_start(x_tiles[0][:, C0:C1], x_flat[0:128, C0:C1])
        nc.scalar.dma_start(x_tiles[0][:, 0:C0], x_flat[0:128, 0:C0])
        for t in range(1, ntiles):
            b0 = t * 128
            nc.scalar.dma_start(x_tiles[t][:, 0:C0], x_flat[b0 : b0 + 128, 0:C0])
            nc.scalar.dma_start(x_tiles[t][:, C0:FD], x_flat[b0 : b0 + 128, C0:FD])

        # ---- phase A: PE + ACT chunk pipeline (both tiles) ----
        xw_tiles = []
        for t in range(ntiles):
            xw_tiles.append(xw_pool.tile([128, FD], DT, name=f"xw{t}"))

        for (t, fp) in CHUNK_ORDER:
            if t >= ntiles:
                continue
            x_t, xw_t = x_tiles[t], xw_tiles[t]
            tr_ps = psum_tr.tile([128, 128], DT)
            nc.tensor.transpose(
                tr_ps[:], x_t[:, fp * 128 : (fp + 1) * 128], ident[:]
            )
            tr_sb = tr_pool.tile([128, 128], BF)   # cast f32->bf16 in the copy
            nc.scalar.copy(tr_sb[:], tr_ps[:])
            mm = psum_mm.tile([128, 128], DT, tag="mm")
            nc.tensor.matmul(mm[:], tr_sb[:], w_bd[:], start=True, stop=True)
            nc.scalar.copy(xw_t[:, fp * 128 : (fp + 1) * 128], mm[:])

        # ---- phase B: DVE muls + per-block DMAs on the sync queue.
        # The last N_END blocks (contiguous in the output row) share ONE
        # stage tile and ONE DMA: the SP sequencer otherwise paces the tail
        # at ~1.4us per DMA issue. ----
        n_blocks = len(BLOCK_ORDER)
        end_blocks = BLOCK_ORDER[n_blocks - N_END :]
        end_i0 = end_blocks[0][1]
        end_w = sum(F - 1 - i for (_, i) in end_blocks)
        end_tile = stage_end.tile([128, end_w * D], DT)
        for k, (t, i) in enumerate(BLOCK_ORDER):
            if t >= ntiles:
                continue
            b0 = t * 128
            x_t, xw_t = x_tiles[t], xw_tiles[t]
            jn = F - 1 - i
            if k >= n_blocks - N_END:
                off = (BLOCK_OFF[i] - BLOCK_OFF[end_i0]) * D
                st = end_tile[:, off : off + jn * D]
            else:
                st = stage_a.tile([128, jn * D], DT, name="st")[:]
            in0 = (
                xw_t[:, i * D : (i + 1) * D]
                .unsqueeze(1)
                .broadcast_to([128, jn, D])
            )
            in1 = x_t[:, (i + 1) * D : FD].rearrange("p (j d) -> p j d", d=D)
            nc.vector.tensor_mul(
                st.rearrange("p (j d) -> p j d", d=D), in0, in1
            )
            if k >= n_blocks - N_END:
                if k == n_blocks - 1:
                    bend = end_blocks[0][0] * 128
                    nc.sync.dma_start(
                        out_dram[
                            bend : bend + 128,
                            BLOCK_OFF[end_i0] * D : (BLOCK_OFF[end_i0] + end_w) * D,
                        ],
                        end_tile[:],
                    )
            else:
                nc.sync.dma_start(
                    out_dram[
                        b0 : b0 + 128,
                        BLOCK_OFF[i] * D : (BLOCK_OFF[i] + jn) * D,
                    ],
                    st,
                )

    nc.compile()
    return nc


def _get_nc(bs: int):
    if bs not in _CACHE:
        _CACHE[bs] = _build(bs)
    return _CACHE[bs]


def _run(inputs: np.ndarray, w: np.ndarray, trace: bool = False):
    inputs = np.ascontiguousarray(inputs, dtype=np.float32)
    w = np.ascontiguousarray(w, dtype=np.float32)
    assert inputs.shape == (B, F, D) and w.shape == (D, D)
    nc = _get_nc(BS)
    ident = np.eye(128, dtype=np.float32)
    wbd = np.zeros((128, 128), dtype=BF_NP)
    wbd[0:D, 0:D] = w.astype(BF_NP)
    wbd[D:128, D:128] = w.astype(BF_NP)
    in_maps = [
        {"x": inputs[c * BS : (c + 1) * BS], "wbd": wbd, "ident": ident}
        for c in range(NCORES)
    ]
    res = run_bass_kernel_spmd(nc, in_maps, list(range(NCORES)), trace=trace)
    out = np.concatenate([res.results[c]["out"] for c in range(NCORES)], axis=0)
    return out, res


def kernel(inputs: np.ndarray, w: np.ndarray) -> np.ndarray:
    out, _ = _run(inputs, w)
    return out
